# revision 35
# baseline (speedup 1.0000x reference)
"""Trainium2 Bass kernel for nn_BaseSelfAttention_88433376625006.

Computes: LayerNorm -> QKV projection -> 12-head causal self-attention
(seq 4096, dim 768) -> output projection, on 8 NeuronCores.

Sharding: 4 teams x 2 cores. Team t owns heads {3t, 3t+1, 3t+2}. Within a
team, core role 0 handles query rows {0..1023, 3072..4095} and role 1 rows
{1024..3071} (equal causal work). Each core computes LN + K/V for the keys
it needs (keys are replicated inside a team), flash-style attention with the
sim matrix in [k, q] layout, and a partial output projection over its heads;
the host scatters rows and sums the 4 team partials. No collectives.

Numerics: matmuls run in float32r (full-rate fp32, ~1.5e-4 rounding);
softmax skips the max-subtraction (sim values are O(1) here) so the
denominator rides the attention matmul as a ones-column of V.
"""

import numpy as np

HEADS = 12
N = 4096
D = 768
DH = 64
LN_EPS = 1e-5
TEAM_HEADS = 3
HD = TEAM_HEADS * DH  # head dims per core = 192

ROLE_SPEC = {
    0: dict(key_rows=4096, q0s=(0, 512, 3072, 3584)),
    1: dict(key_rows=3072, q0s=(1024, 1536, 2048, 2560)),
}

_RUNNERS = None  # lazy build cache
XN_ON_ACT = False
STAGES = "ABC"  # debug: which stages to emit


# --------------------------------------------------------------------------
# neuronxcc workaround: this build rejects instructions with >1 sync wait.
# --------------------------------------------------------------------------
def _install_tile_patch():
    import concourse.tile as tile
    from concourse import mybir
    from concourse.vector_clock import ScopedClock

    if getattr(tile.TileContext, "_single_wait_patch", False):
        return

    def _patched_drain_and_barrier(self, tick_clock, wait_clock):
        nc = self.nc
        probe = nc.sync.nop(nofuse=True, hint="split_drain_waits")
        wait_clock.add_sem_waits(
            probe.ins, ScopedClock({None: tick_clock.global_clock})
        )
        si = probe.ins.sync_info
        waits = list(si.on_wait) if si and si.on_wait else []
        if len(waits) > 1:
            si.on_wait = waits[:1]
            for i in range(1, len(waits)):
                extra = nc.sync.nop(nofuse=True, hint=f"split_drain_waits_{i}")
                xsi = extra.ins.sync_info
                if xsi is None:
                    extra.ins.sync_info = mybir.SyncInfo(
                        on_wait=[waits[i]], on_update=[]
                    )
                else:
                    xsi.on_wait = [waits[i]]
        nc.sync.drain()
        nc.all_engine_barrier()
        popped = nc._tile_sem_poison_stack.pop()
        assert popped is self._sem_poison
        nc.clear_and_free_semaphores(list(self.sems.allocated().values()))
        nc.all_engine_barrier()

    tile.TileContext._drain_and_barrier = _patched_drain_and_barrier

    _orig_commit = tile.TileContext._commit_instruction

    def _patched_commit_instruction(self, inst, lazy_reg_writes=True):
        si = getattr(inst, "sync_info", None)
        if (
            si is not None
            and si.on_wait
            and len(si.on_wait) > 1
            and inst.engine != mybir.EngineType.Unassigned
        ):
            waits = list(si.on_wait)
            si.on_wait = waits[-1:]
            for w in waits[:-1]:
                nop = mybir.InstNoOp(
                    name=self.nc.get_next_instruction_name(),
                    sync_info=mybir.SyncInfo(on_wait=[w], on_update=[]),
                    bass_nofuse=True,
                    engine=inst.engine,
                )
                _orig_commit(self, nop, lazy_reg_writes=False)
        return _orig_commit(self, inst, lazy_reg_writes=lazy_reg_writes)

    tile.TileContext._commit_instruction = _patched_commit_instruction
    tile.TileContext._single_wait_patch = True


# --------------------------------------------------------------------------
# Per-device program dispatch (different programs on different cores).
# --------------------------------------------------------------------------
def _make_runner(nc):
    import jax
    from concourse import mybir
    from concourse.bass2jax import _bass_exec_p, install_neuronx_cc_hook

    install_neuronx_cc_hook()
    pid_name = nc.partition_id_tensor.name if nc.partition_id_tensor else None
    in_names, out_names, out_avals, zero_outs = [], [], [], []
    for alloc in nc.m.functions[0].allocations:
        if not isinstance(alloc, mybir.MemoryLocationSet):
            continue
        name = alloc.memorylocations[0].name
        if alloc.kind == "ExternalInput":
            if name != pid_name:
                in_names.append(name)
        elif alloc.kind == "ExternalOutput":
            shape = tuple(alloc.tensor_shape)
            dtype = mybir.dt.np(alloc.dtype)
            out_names.append(name)
            out_avals.append(jax.core.ShapedArray(shape, dtype))
            zero_outs.append(np.zeros(shape, dtype))
    n_params = len(in_names)
    all_names = in_names + out_names + ([pid_name] if pid_name else [])
    donate = tuple(range(n_params, n_params + len(out_names)))

    def _body(*args):
        return tuple(
            _bass_exec_p.bind(
                *args,
                out_avals=tuple(out_avals),
                in_names=tuple(all_names),
                out_names=tuple(out_names),
                lowering_input_output_aliases=(),
                sim_require_finite=True,
                sim_require_nnan=True,
                nc=nc,
            )
        )

    jitted = jax.jit(_body, donate_argnums=donate, keep_unused=True)
    jitted_nodonate = jax.jit(_body, keep_unused=True)

    def run(in_map, device, core_id=0):
        args = [jax.device_put(np.asarray(in_map[n]), device) for n in in_names]
        args += [jax.device_put(z.copy(), device) for z in zero_outs]
        if pid_name is not None:
            args.append(jax.device_put(np.array([[core_id]], np.uint32), device))
        outs = jitted(*args)
        return {n: outs[i] for i, n in enumerate(out_names)}

    def stage(in_map, device, core_id=0):
        args = [jax.device_put(np.asarray(in_map[n]), device) for n in in_names]
        args += [jax.device_put(z, device) for z in zero_outs]
        if pid_name is not None:
            args.append(jax.device_put(np.array([[core_id]], np.uint32), device))
        return args

    def run_staged(args):
        return jitted_nodonate(*args)

    run.stage = stage
    run.run_staged = run_staged
    run.out_names = out_names
    return run


# --------------------------------------------------------------------------
# The kernel program for one role.
# --------------------------------------------------------------------------
def _build_role_program(role, masked=False):
    import concourse.bass as bass
    import concourse.tile as tile
    from concourse import mybir

    F32 = mybir.dt.float32
    F32R = mybir.dt.float32r
    AF = mybir.ActivationFunctionType
    ALU = mybir.AluOpType

    spec = ROLE_SPEC[role]
    KR = spec["key_rows"]  # key rows this core needs
    q0s = spec["q0s"]  # global start row of each 512-row query tile
    KC = KR // 512  # number of 512-row chunks
    KB = KR // 128  # number of 128-row key blocks
    q_chunks = {q0 // 512: qi for qi, q0 in enumerate(q0s)}  # chunk -> q index
    # attention for q-tile qi can run once chunks <= (q0//512 + 1) are done
    b_after = {}
    for qi, q0 in enumerate(q0s):
        b_after.setdefault(max(q0 // 512, q_chunks and 0) + (1 if q0 // 512 + 1 < KC and (q0 % 512 == 0) else 0), [])
    b_after = {}
    for qi, q0 in enumerate(q0s):
        need = max(q0 // 512, 0)  # straddle blocks live in chunk q0//512
        need = max(need, q0 // 512)
        b_after.setdefault(need, []).append(qi)

    xn_on_act = XN_ON_ACT
    nc = bass.Bass(enable_partition_id=False)

    x_in = nc.declare_dram_parameter("x", [KR, D], F32, isOutput=False)
    wg_in = nc.declare_dram_parameter("wg", [128, 6, 3 * HD], F32R, isOutput=False)
    wv_in = nc.declare_dram_parameter("wvp", [128, 6, 256], F32R, isOutput=False)
    cbv_in = nc.declare_dram_parameter("cbvp", [1, 256], F32R, isOutput=False)
    cb_in = nc.declare_dram_parameter("cb", [1, 3 * HD], F32R, isOutput=False)
    wo_in = nc.declare_dram_parameter("wo", [128, 1536], F32R, isOutput=False)
    mk_in = nc.declare_dram_parameter("maskv", [128, KB], F32, isOutput=False)
    mb_in = nc.declare_dram_parameter("mb", [128, 128], F32R, isOutput=False)
    id_in = nc.declare_dram_parameter("ident", [128, 128], F32R, isOutput=False)
    on_in = nc.declare_dram_parameter("ones", [1, 512], F32R, isOutput=False)
    y_out = nc.declare_dram_parameter("out", [2048, D], F32, isOutput=True)

    with tile.TileContext(nc) as tc:
        with (
            tc.tile_pool(name="persist", bufs=1) as pp,
            tc.tile_pool(name="work", bufs=2) as wk,
            tc.tile_pool(name="xntp", bufs=2) as xp,
            tc.tile_pool(name="xtp", bufs=5) as xtp,
            tc.tile_pool(name="small", bufs=4) as sm,
            tc.tile_pool(name="expp", bufs=4) as ep,
            tc.tile_pool(name="psga", bufs=(3 if role == 0 else 2), space="PSUM") as ps_a,
            tc.tile_pool(name="psim", bufs=2, space="PSUM") as ps_b,
            tc.tile_pool(name="pso", bufs=(1 if role == 0 else 2), space="PSUM") as ps_o,
        ):
            # ---- persistent tiles ----
            ident = pp.tile([128, 128], F32R, tag="ident")
            nc.sync.dma_start(out=ident, in_=id_in[:])
            ones_row = pp.tile([1, 512], F32R, tag="ones_row")
            nc.sync.dma_start(out=ones_row, in_=on_in[:])
            maskv = pp.tile([128, KB], F32, tag="maskv")
            nc.sync.dma_start(out=maskv, in_=mk_in[:])
            mb = pp.tile([128, 128], F32R, tag="mb")
            nc.sync.dma_start(out=mb, in_=mb_in[:])
            cb = pp.tile([1, 3 * HD], F32R, tag="cb")
            nc.sync.dma_start(out=cb, in_=cb_in[:])
            eps_t = pp.tile([128, 1], F32, tag="eps")
            nc.vector.memset(eps_t, LN_EPS)
            wg = pp.tile([128, 6, 3 * HD], F32R, tag="wg")
            nc.gpsimd.dma_start(out=wg, in_=wg_in[:])
            wv_pad = pp.tile([128, 6, 256], F32R, tag="wv_pad")
            nc.gpsimd.dma_start(out=wv_pad, in_=wv_in[:])
            cbv_pad = pp.tile([1, 256], F32R, tag="cbv_pad")
            nc.gpsimd.dma_start(out=cbv_pad, in_=cbv_in[:])
            wo = pp.tile([128, 1536], F32R, tag="wo")
            nc.gpsimd.dma_start(out=wo, in_=wo_in[:])

            # per-chunk / per-qtile persistent tiles => fine-grained deps
            qhh = [
                [pp.tile([128, 512], F32R, name=f"qh{h}_{qi}", tag=f"qh{h}_{qi}") for qi in range(4)]
                for h in range(3)
            ]
            khh = [
                [pp.tile([128, 256], F32R, name=f"kh{h}_{c}", tag=f"kh{h}_{c}") for c in range(KC)]
                for h in range(3)
            ]
            vv = [
                pp.tile([128, 4, 3, 65], F32R, name=f"vv{c}", tag=f"vv{c}")
                for c in range(KC)
            ]
            oq = [pp.tile([128, 512], F32R, name=f"oq{qi}", tag=f"oq{qi}") for qi in range(4)]
            oq2 = [pp.tile([64, 512], F32R, name=f"oq2_{qi}", tag=f"oq2_{qi}") for qi in range(4)]

            # psum->sbuf copies, round-robin with a per-stage ACT share.
            # set_cp(k>0): 1/k of copies on ACT; set_cp(k<0): 1/|k| on DVE.
            _cp_state = [0, 2]

            def cp(out, in_):
                _cp_state[0] += 1
                k = _cp_state[1]
                on_act = (
                    _cp_state[0] % k == 0 if k > 0 else _cp_state[0] % (-k) != 0
                )
                if on_act:
                    nc.scalar.copy(out=out, in_=in_)
                else:
                    nc.vector.tensor_copy(out=out, in_=in_)

            def set_cp(act_every):
                _cp_state[1] = act_every

            # ---------- stage A: LN + transpose + QKV for one 512-row chunk ----
            def stage_a_chunk(c):
                set_cp(-3 if role == 0 else 2)  # stage-A copy balance per role
                xnT = xp.tile([128, 6, 512], F32R, tag="xnT", name=f"xnT{c}")
                x_ts = []
                mvs = sm.tile([128, 4, 2], F32, tag="mvs", name=f"mvs{c}")
                for rb in range(4):
                    row0 = c * 512 + rb * 128
                    x_t = xtp.tile([128, D], F32, tag="x_t", name=f"x{c}_{rb}")
                    x_ts.append(x_t)
                    nc.sync.dma_start(out=x_t, in_=x_in[row0 : row0 + 128, :])
                    xr = x_t.rearrange("p (s f) -> p s f", f=256)
                    st = sm.tile([128, 3, 6], F32, tag="st", name=f"st{c}_{rb}")
                    for s in range(3):
                        nc.vector.bn_stats(out=st[:, s, :], in_=xr[:, s, :])
                    nc.vector.bn_aggr(out=mvs[:, rb, :], in_=st)
                sds = sm.tile([128, 4], F32, tag="sds", name=f"sds{c}")
                rstds = sm.tile([128, 4], F32, tag="rstds", name=f"rss{c}")
                if c == 0:  # latency-critical first chunk: per-rowblock chain
                    for rb in range(4):
                        nc.scalar.activation(
                            out=sds[:, rb : rb + 1], in_=mvs[:, rb, 1:2],
                            func=AF.Sqrt, bias=eps_t, scale=1.0,
                        )
                        nc.vector.reciprocal(
                            out=rstds[:, rb : rb + 1], in_=sds[:, rb : rb + 1]
                        )
                else:
                    nc.scalar.activation(
                        out=sds, in_=mvs[:, :, 1], func=AF.Sqrt, bias=eps_t, scale=1.0
                    )
                    nc.vector.reciprocal(out=rstds, in_=sds)
                if xn_on_act:
                    nmrs = sm.tile([128, 4], F32, tag="nmrs", name=f"nmrs{c}")
                    nc.vector.tensor_scalar(
                        out=nmrs,
                        in0=mvs[:, :, 0],
                        scalar1=-1.0,
                        scalar2=None,
                        op0=ALU.mult,
                    )
                    nc.vector.tensor_mul(out=nmrs, in0=nmrs, in1=rstds)
                for rb in range(4):
                    x_t = x_ts[rb]
                    xn = wk.tile([128, D], F32R, tag="xn", name=f"xn{c}_{rb}")
                    if xn_on_act:
                        with nc.allow_low_precision(reason="xn rounds to f32r"):
                            nc.scalar.activation(
                                out=xn, in_=x_t, func=AF.Identity,
                                bias=nmrs[:, rb : rb + 1],
                                scale=rstds[:, rb : rb + 1],
                            )
                    else:
                        nc.vector.tensor_scalar(
                            out=xn,
                            in0=x_t,
                            scalar1=mvs[:, rb, 0:1],
                            scalar2=rstds[:, rb : rb + 1],
                            op0=ALU.subtract,
                            op1=ALU.mult,
                        )
                    for half in range(2):
                        pt = ps_a.tile([128, 512], F32R, tag="mma", name=f"pt{c}_{rb}_{half}")
                        for dd in range(3):
                            d = 3 * half + dd
                            nc.tensor.transpose(
                                pt[:, dd * 128 : (dd + 1) * 128],
                                xn[:, d * 128 : (d + 1) * 128],
                                ident,
                            )
                        cp(
                            xnT[:, 3 * half : 3 * half + 3, rb * 128 : (rb + 1) * 128],
                            pt[:, 0:384].rearrange("p (t f) -> p t f", f=128),
                        )

                qi = q_chunks.get(c)
                if qi is not None:
                    groups = [(0, 128), (128, 256), (256, 384)]
                else:
                    groups = [(192, 320), (320, 384)]
                for g0, g1 in groups:
                    gp = ps_a.tile([g1 - g0, 512], F32, tag="mma", name=f"gp{c}_{g0}")
                    for d in range(6):
                        nc.tensor.matmul(
                            gp, wg[:, d, g0:g1], xnT[:, d, :], start=(d == 0), stop=False
                        )
                    nc.tensor.matmul(gp, cb[:, g0:g1], ones_row, start=False, stop=True)
                    for s64 in range(g0, g1, 64):
                        kind, h = s64 // 192, (s64 % 192) // 64
                        sub = gp[s64 - g0 : s64 - g0 + 64, :]
                        if kind == 0:  # q, duplicated across partition halves
                            cp(qhh[h][qi][0:64, :], sub)
                            cp(qhh[h][qi][64:128, :], sub)
                        elif kind == 1:  # kT arranged by block parity
                            sub4 = sub.rearrange("p (t f) -> p t f", f=128)
                            cp(
                                khh[h][c][0:64, :].rearrange("p (t f) -> p t f", f=128),
                                sub4[:, 0::2, :],
                            )
                            cp(
                                khh[h][c][64:128, :].rearrange("p (t f) -> p t f", f=128),
                                sub4[:, 1::2, :],
                            )
                        else:
                            raise AssertionError("v handled separately")
                # V in natural [key, dim] layout: xnT tiles as stationary
                for rb in range(4):
                    pvn = ps_a.tile([128, 256], F32, tag="mma", name=f"pvn{c}_{rb}")
                    for d in range(6):
                        nc.tensor.matmul(
                            pvn,
                            xnT[:, d, rb * 128 : (rb + 1) * 128],
                            wv_pad[:, d, :],
                            start=(d == 0),
                            stop=False,
                        )
                    nc.tensor.matmul(
                        pvn, ones_row[:, 0:128], cbv_pad, start=False, stop=True
                    )
                    if masked:
                        nc.vector.tensor_scalar_mul(
                            out=vv[c][:, rb, :, 0:64].rearrange("p h f -> p (h f)"),
                            in0=pvn[:, 0:192],
                            scalar1=maskv[:, 4 * c + rb : 4 * c + rb + 1],
                        )
                    else:
                        cp(vv[c][:, rb, :, 0:64], pvn[:, 0:192].rearrange("p (h f) -> p h f", f=64))
                for h in range(3):
                    nc.gpsimd.tensor_copy(
                        out=vv[c][:, :, h, 64], in_=maskv[:, 4 * c : 4 * c + 4]
                    )

            # ---------- stage B: attention for one (head, q-tile) ----------
            def stage_b(h, qi):
                set_cp(4 if role == 0 else 3)  # B copies mostly on DVE
                q0 = q0s[qi]
                po = ps_o.tile([65, 512], F32, tag="po", name=f"po{h}_{qi}")
                qsl_lo = qhh[h][qi][0:64, :]
                qsl_hi = qhh[h][qi][64:128, :]
                first = True
                npairs = q0 // 256
                for p in range(npairs):
                    kb0 = 2 * p
                    c0 = 128 * p
                    pe_ = ps_b.tile([128, 1024], F32, tag="mmb", name=f"sp{h}_{qi}_{p}")
                    kc, kcol = p // 2, 128 * (p % 2)
                    nc.tensor.matmul(
                        pe_[:, 0:512], khh[h][kc][0:64, kcol : kcol + 128], qsl_lo,
                        start=True, stop=True,
                    )
                    nc.tensor.matmul(
                        pe_[:, 512:1024], khh[h][kc][64:128, kcol : kcol + 128], qsl_hi,
                        start=True, stop=True,
                    )
                    ee = ep.tile([128, 1024], F32R, tag="exp", name=f"ee{h}_{qi}_{p}")
                    nc.scalar.activation(out=ee, in_=pe_, func=AF.Exp)
                    nc.tensor.matmul(
                        po, vv[kb0 // 4][:, kb0 % 4, h, :], ee[:, 0:512],
                        start=first, stop=False,
                    )
                    first = False
                    nc.tensor.matmul(
                        po, vv[(kb0 + 1) // 4][:, (kb0 + 1) % 4, h, :], ee[:, 512:1024],
                        start=False, stop=False,
                    )
                # straddles: s0(512)+s1(384)+s3(128) packed in ps1; s2(256) in ps2
                kbase = q0 // 128
                ps1 = ps_b.tile([128, 1024], F32, tag="mmb", name=f"s1_{h}_{qi}")
                ps2 = ps_b.tile([128, 1024], F32, tag="mmb", name=f"s2_{h}_{qi}")
                placing = [(ps1, 0), (ps1, 512), (ps2, 0), (ps1, 896)]
                for si, (dstp, o0) in enumerate(placing):
                    r = 128 * si
                    ns = 512 - r
                    kb = kbase + si
                    phalf = 64 * (kb % 2)
                    pcol = 128 * (kb // 2)
                    qsl = qhh[h][qi][phalf : phalf + 64, r:512]
                    kc, kcol = kb // 4, 128 * ((kb % 4) // 2)
                    nc.tensor.matmul(
                        dstp[:, o0 : o0 + ns],
                        khh[h][kc][phalf : phalf + 64, kcol : kcol + 128],
                        qsl,
                        start=True, stop=True, skip_group_check=True,
                    )
                es1 = ep.tile([128, 1024], F32R, tag="exp", name=f"e1_{h}_{qi}")
                es2 = ep.tile([128, 1024], F32R, tag="exp", name=f"e2_{h}_{qi}")
                nc.scalar.activation(out=es1, in_=ps1, func=AF.Exp)
                nc.scalar.activation(out=es2[:, 0:256], in_=ps2[:, 0:256], func=AF.Exp)
                epl = [(es1, 0), (es1, 512), (es2, 0), (es1, 896)]
                for es, o0 in epl:
                    nc.gpsimd.tensor_mul(
                        out=es[:, o0 : o0 + 128], in0=es[:, o0 : o0 + 128], in1=mb
                    )
                for si, (es, o0) in enumerate(epl):
                    r = 128 * si
                    ns = 512 - r
                    kb = kbase + si
                    nc.tensor.matmul(
                        po[:, r:512],
                        vv[kb // 4][:, kb % 4, h, :],
                        es[:, o0 : o0 + ns],
                        start=first, stop=(si == 3),
                    )
                    first = False
                # normalize by denominator (row 64)
                rden = sm.tile([1, 512], F32R, tag="rden", name=f"rd{h}_{qi}")
                with nc.allow_low_precision(reason="recip feeds PE broadcast"):
                    nc.vector.reciprocal(out=rden, in_=po[64:65, :])
                rdp = ps_a.tile([64, 512], F32, tag="mma", name=f"rdp{h}_{qi}")
                nc.tensor.matmul(rdp, ones_row[:, 0:64], rden, start=True, stop=True)
                rdb = sm.tile([64, 512], F32, tag="rdb", name=f"rdb{h}_{qi}")
                nc.vector.tensor_copy(out=rdb, in_=rdp)
                dst = oq[qi][64 * h : 64 * h + 64, :] if h < 2 else oq2[qi]
                nc.vector.tensor_tensor(out=dst, in0=po[0:64, :], in1=rdb, op=ALU.mult)

            # ---------- stage C: output projection for one q-tile ----------
            def stage_c(qi):
                for rbl in range(4):
                    rb = 4 * qi + rbl
                    a_sl = oq[qi][:, rbl * 128 : (rbl + 1) * 128]
                    b_sl = oq2[qi][:, rbl * 128 : (rbl + 1) * 128]
                    py = ps_b.tile([128, 1024], F32, tag="mmb", name=f"py{rb}")
                    nc.tensor.matmul(py[:, 0:512], a_sl, wo[:, 0:512], start=True, stop=False)
                    nc.tensor.matmul(py[:, 0:512], b_sl, wo[0:64, 768:1280], start=False, stop=True)
                    nc.tensor.matmul(py[:, 512:768], a_sl, wo[:, 512:768], start=True, stop=False)
                    nc.tensor.matmul(py[:, 512:768], b_sl, wo[0:64, 1280:1536], start=False, stop=True)
                    y_sb = wk.tile([128, D], F32, tag="y_sb", name=f"y{rb}")
                    cp(y_sb, py[:, 0:768])
                    nc.sync.dma_start(out=y_out[rb * 128 : (rb + 1) * 128, :], in_=y_sb)

            # ---------- emission: interleave B/C into the A chunk loop ----------
            for c in range(KC):
                if "A" in STAGES:
                    stage_a_chunk(c)
                for qi in b_after.get(c, []):
                    if "B" in STAGES:
                        for h in range(3):
                            stage_b(h, qi)
                    if "C" in STAGES:
                        stage_c(qi)

    return nc


# --------------------------------------------------------------------------
# Host-side input prep
# --------------------------------------------------------------------------
def _prep_inputs(x, ln_g, ln_b, w_qkv, w_out, mask):
    x2d = np.asarray(x, np.float32).reshape(N, D)
    ln_g = np.asarray(ln_g, np.float32)
    ln_b = np.asarray(ln_b, np.float32)
    w_qkv = np.asarray(w_qkv, np.float32)
    w_out = np.asarray(w_out, np.float32)
    maskf = np.asarray(mask).reshape(N).astype(np.float32)
    scale = DH ** -0.5

    inner = HEADS * DH
    wq, wk_, wv = w_qkv[:, :inner], w_qkv[:, inner : 2 * inner], w_qkv[:, 2 * inner :]
    weff_q = (ln_g[:, None] * wq) * scale
    weff_k = ln_g[:, None] * wk_
    weff_v = ln_g[:, None] * wv
    cb_q = (ln_b @ wq) * scale
    cb_k = ln_b @ wk_
    cb_v = ln_b @ wv

    mb = np.triu(np.ones((128, 128), np.float32))
    ident = np.eye(128, dtype=np.float32)

    per_core = []
    for c in range(8):
        t, role = divmod(c, 2)
        spec = ROLE_SPEC[role]
        KR = spec["key_rows"]
        KB = KR // 128
        hsl = slice(3 * t * DH, (3 * t + 3) * DH)
        # [q|k|v] effective weights for this team's heads: [768, 576]
        wcat = np.concatenate(
            [weff_q[:, hsl], weff_k[:, hsl], weff_v[:, hsl]], axis=1
        )
        wg = np.ascontiguousarray(
            wcat.reshape(6, 128, 3 * HD).transpose(1, 0, 2)
        )  # [128, 6, 576]
        wvp = np.zeros((128, 6, 256), np.float32)
        wvp[:, :, 0:192] = weff_v[:, hsl].reshape(6, 128, HD).transpose(1, 0, 2)
        cbvp = np.zeros((1, 256), np.float32)
        cbvp[0, 0:192] = cb_v[hsl]
        cb = np.concatenate([cb_q[hsl], cb_k[hsl], cb_v[hsl]])[None, :]
        wo_t = w_out[hsl, :]  # [192, 768]
        wo_packed = np.zeros((128, 1536), np.float32)
        wo_packed[:, :768] = wo_t[:128]
        wo_packed[:64, 768:] = wo_t[128:]
        maskv = np.ascontiguousarray(maskf[:KR].reshape(KB, 128).T)  # [128, KB]
        per_core.append(
            dict(
                x=np.ascontiguousarray(x2d[:KR]),
                wg=wg,
                cb=np.ascontiguousarray(cb),
                wo=wo_packed,
                maskv=maskv,
                mb=mb,
                ident=ident,
                ones=np.ones((1, 512), np.float32),
                wvp=wvp,
                cbvp=cbvp,
            )
        )
    return per_core


def _get_runners(masked=False):
    global _RUNNERS
    if _RUNNERS is None or _RUNNERS[2] != masked:
        _install_tile_patch()
        _RUNNERS = [
            _make_runner(_build_role_program(0, masked)),
            _make_runner(_build_role_program(1, masked)),
            masked,
        ]
    return _RUNNERS


def kernel(x, ln_g, ln_b, w_qkv, w_out, mask):
    import jax

    runners = _get_runners(masked=not np.asarray(mask).all())
    per_core = _prep_inputs(x, ln_g, ln_b, w_qkv, w_out, mask)
    devs = jax.devices()
    futs = [
        runners[c % 2](per_core[c], devs[c], core_id=c) for c in range(8)
    ]
    outs = [np.asarray(f["out"]) for f in futs]

    full = np.zeros((N, D), np.float32)
    for t in range(4):
        for role in (0, 1):
            o = outs[2 * t + role]
            for qi, q0 in enumerate(ROLE_SPEC[role]["q0s"]):
                full[q0 : q0 + 512] += o[qi * 512 : (qi + 1) * 512]
    return full.reshape(np.asarray(x).shape).astype(np.float32)
